# revision 29
# baseline (speedup 1.0000x reference)
"""PSANet 'distribute' gather kernel for Trainium2 (8 NeuronCores, SPMD).

Problem:
    x: (N=2, 16129=127*127, H=64, W=64) f32
    out[n, h*64+w, i, j] = x[n, (i-h+63)*127 + (j-w+63), h, w]

Sharding: over the h part of the output-channel dim (h*64+w): core k owns
h in [8k, 8k+8).  Per-core input is a pure numpy slice of x along
(channel, h); output shards concatenate along the channel dim.

Per-core kernel (same program on all cores; the host pre-shifts the
channel window so the program is core-independent):
    xs[n, pl*127+q, hl, w] = x[n, (pl+56-h0)*127+q, h0+hl, w],  pl in [0,71)
    For each (n, hl):
      - DMA X tile [64 part = p-window [7-hl,71-hl), 127q x 64w]
        (3-dim AP, 256B chunks; n=0 on the SP HWDGE ring, n=1 on the ACT
        ring - the two rings drain concurrently).
      - Engine gather (DVE / GpSimd alternating by hl):
        ot[i, w*64+j] = xt[i, (j+63-w)*64 + w]   (strides -63 / +64)
      - DMA store ot -> os[n, hl*64+w, i, j] on the opposite ring.

Measured notes (this toolchain/axon setup):
  * DMA APs are hard-capped at 3 dims; 64-partition 3-dim loads are the
    fast path.  71/128-partition or 4-dim variants fall off a
    descriptor-generation cliff (5-10x slower).
  * Splitting traffic across both HWDGE rings (sync+scalar) is ~1.55x
    faster than one ring; SWDGE (gpsimd) stores are much slower.
"""

import numpy as np

N, H, W = 2, 64, 64
Q = 2 * W - 1          # 127
PW = 71                # per-core p-window width (union over 8 h values)
HL = 8                 # h values per core
NCORES = 8

_cache = {}


def _build_bass(repeat=1, stage=3, xbufs=4, obufs=3):
    import concourse.bass as bass
    import concourse.mybir as mybir
    from concourse.tile import TileContext

    def _split_multi_waits():
        """This container's walrus accepts at most ONE sync-wait per
        instruction; Tile's wait assignment can attach several.  Hoist
        extra waits onto NOPs inserted right before the instruction on
        the same engine (sequencers execute waits in program order, so
        semantics are identical)."""
        for fn in nc.m.functions:
            for blk in fn.blocks:
                old = blk.instructions
                new = []
                changed = False
                for inst in old:
                    si = inst.sync_info
                    waits = list(si.on_wait) if si is not None and si.on_wait else []
                    if len(waits) > 1:
                        changed = True
                        for wdesc in waits[:-1]:
                            nop = mybir.InstNoOp(
                                name=nc.get_next_instruction_name(), ins=[], outs=[]
                            )
                            nop.engine = inst.engine
                            nop.sync_info = mybir.SyncInfo(
                                on_wait=[wdesc], on_update=list()
                            )
                            new.append(nop)
                        si.on_wait = [waits[-1]]
                        inst.sync_info = si
                    new.append(inst)
                if changed:
                    blk.instructions = new

    f32 = mybir.dt.float32
    nc = bass.Bass(trn_type="TRN2")
    xs = nc.dram_tensor("xs", [N, PW * Q, HL, W], f32, kind="ExternalInput")
    os = nc.dram_tensor("os", [N, HL * W, H, W], f32, kind="ExternalOutput")

    with TileContext(nc) as tc:
        with (
            tc.tile_pool(name="xpool", bufs=xbufs) as xpool,
            tc.tile_pool(name="opool", bufs=obufs) as opool,
        ):
            for _rep in range(repeat):
                # Software-pipelined emission: load(t) ... copy+store(t-2).
                # Each n owns ONE ring for both its loads and stores; the
                # lag interleaves stores between loads so a store (waiting
                # on its copy) never clusters ahead of later loads in the
                # in-order sequencer FIFO, and the two rings run the two
                # n-streams fully concurrently (no phase serialization).
                LAG = 2
                tiles = [(n, hl) for n in range(N) for hl in range(HL)]
                state = {}
                for t in range(len(tiles) + LAG):
                    if t < len(tiles):
                        n, hl = tiles[t]
                        ring = nc.sync if n == 0 else nc.scalar
                        xt = xpool.tile([64, Q * W], f32, name="xt")
                        base = (
                            n * PW * Q + (7 - hl) * Q
                        ) * HL * W + hl * W
                        src = bass.AP(
                            tensor=xs,
                            offset=base,
                            ap=[[Q * HL * W, 64], [HL * W, Q], [1, W]],
                        )
                        ring.dma_start(out=xt[:, :], in_=src)
                        state[t] = (n, hl, xt)
                    td = t - LAG
                    if 0 <= td < len(tiles) and stage >= 2:
                        n, hl, xt = state.pop(td)
                        ring = nc.sync if n == 0 else nc.scalar
                        # gather: ot[i, w*64+j] = xt[i, (j+63-w)*64 + w]
                        ot = opool.tile([64, H * W], f32, name="ot")
                        tv = xt[:, :]
                        gsrc = bass.AP(
                            tensor=tv.tensor,
                            offset=tv.offset + 63 * W,
                            ap=[list(tv.ap[0]), [1 - W, W], [W, W]],
                        )
                        ov = ot[:, :]
                        odst = bass.AP(
                            tensor=ov.tensor,
                            offset=ov.offset,
                            ap=[list(ov.ap[0]), [W, W], [1, W]],
                        )
                        ceng = nc.vector if hl % 2 == 0 else nc.gpsimd
                        ceng.tensor_copy(out=odst, in_=gsrc)
                        if stage >= 3:
                            hdst = bass.AP(
                                tensor=os,
                                offset=(n * HL * W + hl * W) * H * W,
                                ap=[[W, H], [H * W, W], [1, W]],
                            )
                            ring.dma_start(out=hdst, in_=ov)
    _split_multi_waits()
    return nc


def kernel(x):
    from concourse import bass_utils

    x = np.ascontiguousarray(np.asarray(x, dtype=np.float32))
    assert x.shape == (N, Q * Q, H, W), x.shape

    if "nc" not in _cache:
        _cache["nc"] = _build_bass()
    nc = _cache["nc"]

    in_maps = []
    for k in range(NCORES):
        h0 = HL * k
        c0 = (56 - h0) * Q
        in_maps.append(
            {"xs": np.ascontiguousarray(x[:, c0 : c0 + PW * Q, h0 : h0 + HL, :])}
        )

    res = bass_utils.run_bass_kernel_spmd(nc, in_maps, core_ids=list(range(NCORES)))
    out = np.concatenate([r["os"] for r in res.results], axis=1)
    return out
